# revision 26
# baseline (speedup 1.0000x reference)
"""GAT (2-layer graph attention network) Bass kernel for 8 trn2 NeuronCores.

Sharding: core c owns node rows [512c, 512c+512). Weights replicated.
Scores live in transposed layout [j(partitions), i(free)] so the
aggregation matmul out1T[d', i] = sum_j h_aug[j, d'] * P[j, i] needs no
on-device transposes; the softmax denominator comes from a ones column
in the augmented feature matrix.

v5: h1/s_dst/s_src are host-precomputed. Layer-1 raw scores are built
ON THE TENSOR ENGINE as a K=5 matmul straight into PSUM:
  lhsT = [s_dst[j,0..3]; 1]  (5 x 128 chunk of host tensor)
  rhs  = [e_h blocks; s_src] (5 x H*SH constant)
  out[j,(h,i)] = s_dst[j,h] + s_src[h,i]
Exp (patched table: exp(lrelu)) reads the PSUM half-chunks directly and
the {0,1} mask applies as one bf16 multiply on DVE - so the vector
engine does a single op per chunk and never fights the PE for SBUF.
Layer 2 keeps the scalar_tensor_tensor form with an additive mask and
gathers the per-shard h2 (f32) in a blocked layout so the post-
collective reload is 8 contiguous DMAs.
"""

import os

import numpy as np

N, FIN, HID, H, D1, C = 4096, 512, 256, 4, 64, 64
NCORES = 8
SH = N // NCORES          # 512 local nodes per core
NB = N // 128             # 32 j-chunks
KC2 = HID // 128          # 2 hid chunks
NEG = 0.2                 # leaky relu slope
AUG = (D1 + 1) * H        # 260: per head [h (64) | 1]
MB = -30000.0             # additive mask value for non-edges (layer 2)
IB = SH // 128            # 4 local i-blocks

_CACHED = {}


def _make_act_root(alpha=NEG):
    """Patch the neuron ACT tables so Exp computes g(x)=exp(lrelu(x)).

    Bucket entries are [d0,d1,d2,d3,x0,0,0,0] fp32 cubics evaluated as
    y = d0+(x-x0)(d1+(x-x0)(d2+(x-x0)d3)). For exp buckets centered at
    x0<0 we substitute the Taylor cubic of exp(alpha*x) at the same
    center; the alpha contraction makes the cubic far more accurate than
    the original spline tolerance. Verified on HW: max rel err ~1.1e-5.
    """
    import json
    import shutil
    import tempfile

    from neuronxcc.driver.Job import Job
    from neuronxcc.driver.jobs.support.FindActInfo import findActInfoFile

    src_dir = os.path.dirname(findActInfoFile(Job.getPackageDir(), "gen3"))
    dst = tempfile.mkdtemp(prefix="gat_act_root_")
    for f in os.listdir(src_dir):
        shutil.copy(os.path.join(src_dir, f), os.path.join(dst, f))
        os.chmod(os.path.join(dst, f), 0o644)
    for set_name in ("exp_and_others", "natural_log_exp_and_others",
                     "exp_and_friends"):
        meta = json.load(open(os.path.join(dst, f"{set_name}.json")))
        start = meta["func_to_bkt_start_idx"].get("exp")
        if start is None:
            continue
        nxt = [s for s in sorted(meta["func_to_bkt_start_idx"].values())
               if s > start]
        end = nxt[0] if nxt else meta["bkt_entry_cnt"]
        path = os.path.join(dst, f"{set_name}_bkt.bin")
        b = np.fromfile(path, dtype=np.float32).reshape(-1, 8).copy()
        for i in range(start, end):
            x0, d0 = float(b[i, 4]), float(b[i, 0])
            if x0 >= 0 or not np.isfinite(d0) or d0 <= 0:
                continue
            e = np.exp(alpha * x0)
            b[i, 0:4] = [e, alpha * e, alpha * alpha * e / 2.0,
                         alpha ** 3 * e / 6.0]
        b.tofile(path)
    return os.path.join(dst, "act_info.json")


def _build_nc():
    os.environ["BASS_ACT_ROOT_JSON_PATH"] = _make_act_root()
    import concourse.mybir as mybir
    import concourse.tile as tile
    from concourse import bacc

    f32 = mybir.dt.float32
    bf16 = mybir.dt.bfloat16
    Af = mybir.ActivationFunctionType
    Alu = mybir.AluOpType

    nc = bacc.Bacc("TRN2", target_bir_lowering=False, debug=False,
                   num_devices=NCORES)

    # blocked host layouts: partition p holds j = 128*jc + p
    h1b_d = nc.dram_tensor("h1b", [128, NB * AUG], bf16,
                           kind="ExternalInput").ap()
    sdt2_d = nc.dram_tensor("sdt2", [128, N], bf16,
                            kind="ExternalInput").ap()
    srhs_d = nc.dram_tensor("srhs", [128, SH], bf16,
                            kind="ExternalInput").ap()
    mm_d = nc.dram_tensor("maskM", [N, SH], bf16, kind="ExternalInput").ap()
    mb_d = nc.dram_tensor("maskB", [N, SH], bf16, kind="ExternalInput").ap()
    W2e_d = nc.dram_tensor("W2e", [HID, C + 1], bf16, kind="ExternalInput").ap()
    v2s_d = nc.dram_tensor("v2s", [HID, 1], bf16, kind="ExternalInput").ap()
    outT_d = nc.dram_tensor("outT", [C, SH], f32, kind="ExternalOutput").ap()

    with tile.TileContext(nc) as tc:
        with tc.tile_pool(name="persist", bufs=1) as pp:
            h1aug = pp.tile([128, NB, AUG], bf16)
            mmul = pp.tile([128, NB, SH], bf16)     # {0,1} mask (layer 1)
            mbr = pp.tile([128, NB, SH], bf16)      # additive mask (layer 2)
            sdt2 = pp.tile([128, NB, 128], bf16)
            srhs = pp.tile([128, SH], bf16)
            z1Tl = pp.tile([128, KC2, SH], bf16)
            h2f = pp.tile([128, NB, C + 1], f32)    # gathered h2|s2dst
            h2s = pp.tile([128, NB, C + 1], bf16)   # bf16 stationary copy
            s2dstf = pp.tile([128, NB], f32)
            s2srcb = pp.tile([128, SH], bf16)
            s2srow = pp.tile([1, SH], bf16)
            W2sb = pp.tile([128, KC2, C + 1], bf16)
            v2sb = pp.tile([128, KC2, 1], bf16)

            # ---------- startup DMAs -----------------------------------
            nc.sync.dma_start(sdt2[:].rearrange("p n x -> p (n x)"), sdt2_d)
            nc.sync.dma_start(srhs[:], srhs_d)
            for kc in range(KC2):
                nc.sync.dma_start(W2sb[:, kc, :], W2e_d[kc * 128:(kc + 1) * 128, :])
                nc.sync.dma_start(v2sb[:, kc, :], v2s_d[kc * 128:(kc + 1) * 128, :])

            LOOK = 4  # chunks of DMA lookahead

            def fetch(nb):
                nc.sync.dma_start(h1aug[:, nb, :],
                                  h1b_d[:, nb * AUG:(nb + 1) * AUG])
                nc.sync.dma_start(mmul[:, nb, :],
                                  mm_d[nb * 128:(nb + 1) * 128, :])

            for i in range(LOOK):
                fetch(i)

            # ---------- layer 1: masked softmax + aggregation -----------
            with (tc.tile_pool(name="aggps", bufs=1, space="PSUM") as aggps,
                  tc.tile_pool(name="scps", bufs=2, space="PSUM") as scps,
                  tc.tile_pool(name="spool", bufs=2) as sp):
                o1 = aggps.tile([D1 + 1, H, SH], f32)

                for i in range(NB):
                    if i + LOOK < NB:
                        fetch(i + LOOK)
                    pex = sp.tile([128, H, SH], bf16, tag="pex")
                    for half in range(2):
                        sc = scps.tile([128, 2 * SH], f32, tag="sc")
                        for hh in range(2):
                            h = 2 * half + hh
                            nc.tensor.matmul(
                                sc[:, hh * SH:(hh + 1) * SH],
                                sdt2[32 * h:32 * h + 2, i, :],
                                srhs[32 * h:32 * h + 2, :],
                                start=True, stop=True,
                                tile_position=(32 * h, 0))
                        nc.scalar.activation(
                            pex[:, 2 * half:2 * half + 2, :].rearrange(
                                "p h x -> p (h x)"),
                            sc[:], Af.Exp)
                    pt = sp.tile([128, H, SH], bf16, tag="pt")
                    nc.vector.tensor_mul(
                        pt[:], pex[:],
                        mmul[:, i, :].unsqueeze(1).to_broadcast(
                            (128, H, SH)))
                    for h in range(H):
                        nc.tensor.matmul(
                            o1[:, h, :],
                            h1aug[:, i, (D1 + 1) * h:(D1 + 1) * (h + 1)],
                            pt[:, h, :],
                            start=(i == 0), stop=(i == NB - 1))

                # layer-2 additive mask loads overlap the tail/collective
                for i in range(NB):
                    nc.sync.dma_start(mbr[:, i, :],
                                      mb_d[i * 128:(i + 1) * 128, :])

                # ---- normalize + ELU -> z1Tl [256(=2x128), SH] ---------
                with tc.tile_pool(name="fin1", bufs=1) as fin:
                    drow = fin.tile([1, H, SH], f32, tag="drow")
                    nc.vector.tensor_copy(drow[:], o1[D1:D1 + 1, :, :])
                    denb = fin.tile([D1, H, SH], f32, tag="denb")
                    nc.gpsimd.partition_broadcast(
                        denb[:].rearrange("p h x -> p (h x)"),
                        drow[:].rearrange("p h x -> p (h x)"))
                    recb = fin.tile([D1, H, SH], f32, tag="recb")
                    nc.vector.reciprocal_approx_fast(
                        recb[:].rearrange("p h x -> p (h x)"),
                        denb[:].rearrange("p h x -> p (h x)"))
                    for h in range(H):
                        r0 = (h % 2) * D1
                        nc.vector.tensor_mul(z1Tl[r0:r0 + D1, h // 2, :],
                                             o1[0:D1, h, :], recb[:, h, :])
                    for kc in range(KC2):
                        r_ = fin.tile([128, SH], bf16, tag="relu")
                        m_ = fin.tile([128, SH], bf16, tag="minv")
                        e_ = fin.tile([128, SH], bf16, tag="expv")
                        nc.vector.tensor_scalar_max(r_[:], z1Tl[:, kc, :], 0.0)
                        nc.vector.tensor_scalar_min(m_[:], z1Tl[:, kc, :], 0.0)
                        nc.scalar.activation(e_[:], m_[:], Af.Exp, scale=5.0)
                        nc.vector.scalar_tensor_tensor(
                            z1Tl[:, kc, :], e_[:], -1.0, r_[:],
                            op0=Alu.add, op1=Alu.add)

            # ---------- local h2 shard + all-gather ---------------------
            with (tc.tile_pool(name="l2ps", bufs=2, space="PSUM") as l2ps,
                  tc.tile_pool(name="l2sb", bufs=1) as l2sb,
                  tc.tile_pool(name="dram", bufs=1, space="DRAM") as dpool):
                s2p = l2ps.tile([1, SH], f32, tag="s2p")
                for kc in range(KC2):
                    nc.tensor.matmul(s2p[:], v2sb[:, kc, :], z1Tl[:, kc, :],
                                     start=(kc == 0), stop=(kc == KC2 - 1))
                nc.vector.tensor_copy(s2srow[:], s2p[:])
                nc.gpsimd.partition_broadcast(s2srcb[:], s2srow[:])

                h2loc = l2sb.tile([128, IB, C + 1], f32, tag="h2loc")
                for ib in range(IB):
                    h2p = l2ps.tile([128, C + 1], f32, tag="h2p")
                    for kc in range(KC2):
                        nc.tensor.matmul(
                            h2p[:], z1Tl[:, kc, ib * 128:(ib + 1) * 128],
                            W2sb[:, kc, :],
                            start=(kc == 0), stop=(kc == KC2 - 1))
                    nc.vector.tensor_copy(h2loc[:, ib, :], h2p[:])

                # blocked gather payload: [128, IB*(C+1)] contiguous rows
                ag_in = dpool.tile([128, IB * (C + 1)], f32)
                ag_out = dpool.tile([NCORES * 128, IB * (C + 1)], f32,
                                    addr_space="Shared")
                nc.sync.dma_start(ag_in[:],
                                  h2loc[:].rearrange("p b c -> p (b c)"))
                nc.gpsimd.collective_compute(
                    "AllGather", Alu.bypass,
                    replica_groups=[list(range(NCORES))],
                    ins=[ag_in[:].opt()], outs=[ag_out[:].opt()])
                for r in range(NCORES):
                    nc.sync.dma_start(
                        h2f[:, IB * r:IB * (r + 1), :].rearrange(
                            "p b c -> p (b c)"),
                        ag_out[r * 128:(r + 1) * 128, :])
                nc.vector.tensor_copy(s2dstf[:, :], h2f[:, :, C])
                nc.vector.tensor_copy(h2s[:, :, 0:C], h2f[:, :, 0:C])
                nc.vector.memset(h2s[:, :, C:C + 1], 1.0)

            # ---------- layer 2: masked softmax + aggregation -----------
            with (tc.tile_pool(name="aggps2", bufs=1, space="PSUM") as aggps2,
                  tc.tile_pool(name="sp2", bufs=2) as sp2):
                o2 = aggps2.tile([C + 1, SH], f32)
                NQ = NB // 4
                for q in range(NQ):
                    squad = sp2.tile([128, 4, SH], bf16, tag="sq2")
                    pexq = sp2.tile([128, 4, SH], bf16, tag="px2")
                    for k in range(4):
                        i = 4 * q + k
                        nc.vector.scalar_tensor_tensor(
                            squad[:, k, :], s2srcb[:],
                            s2dstf[:, i:i + 1], mbr[:, i, :],
                            op0=Alu.add, op1=Alu.add)
                    nc.scalar.activation(
                        pexq[:].rearrange("p a x -> p (a x)"),
                        squad[:].rearrange("p a x -> p (a x)"),
                        Af.Exp)
                    for k in range(4):
                        i = 4 * q + k
                        nc.tensor.matmul(o2[:], h2s[:, i, :], pexq[:, k, :],
                                         start=(i == 0), stop=(i == NB - 1))

                with tc.tile_pool(name="fin2", bufs=1) as fin2:
                    drow2 = fin2.tile([1, SH], f32, tag="drow2")
                    nc.vector.tensor_copy(drow2[:], o2[C:C + 1, :])
                    denb2 = fin2.tile([C, SH], f32, tag="denb2")
                    nc.gpsimd.partition_broadcast(denb2[:], drow2[:])
                    recb2 = fin2.tile([C, SH], f32, tag="recb2")
                    nc.vector.reciprocal_approx_fast(recb2[:], denb2[:])
                    outsb = fin2.tile([C, SH], f32, tag="outsb")
                    nc.vector.tensor_mul(outsb[:], o2[0:C, :], recb2[:])
                    nc.sync.dma_start(outT_d, outsb[:])

    nc.compile()
    return nc


def _get_nc():
    if "nc" not in _CACHED:
        _CACHED["nc"] = _build_nc()
    return _CACHED["nc"]


def _prep_in_maps(x, A, W1, a1_src, a1_dst, W2, a2_src, a2_dst):
    import ml_dtypes
    bf = ml_dtypes.bfloat16
    f = np.float32
    x = x.astype(f, copy=False)
    W1r = W1.reshape(FIN, H, D1)
    V1s = np.einsum("fhd,hd->fh", W1r, a1_src).astype(f)
    V1d = np.einsum("fhd,hd->fh", W1r, a1_dst).astype(f)
    h1 = x @ W1                       # [N, HID]
    sdstA = x @ V1d                   # [N, H]
    ssrcA = x @ V1s                   # [N, H]
    h1aug = np.empty((N, H, D1 + 1), f)
    h1aug[:, :, 0:D1] = h1.reshape(N, H, D1)
    h1aug[:, :, D1] = 1.0
    h1b = np.ascontiguousarray(
        h1aug.reshape(NB, 128, AUG).transpose(1, 0, 2).reshape(128, NB * AUG)
    ).astype(bf)
    # row-tiled score matmuls: head h occupies PE rows 32h..32h+1 with
    # lhsT = [sdst[:,h]; 1] and rhs = [1; s_src[h,:]]
    sdt2 = np.zeros((128, N), f)
    for h in range(H):
        sdt2[32 * h, :] = sdstA[:, h]
        sdt2[32 * h + 1, :] = 1.0
    sdt2 = sdt2.astype(bf)
    W2e = np.ascontiguousarray(
        np.concatenate([W2, W2 @ a2_dst.T], axis=1)).astype(bf)
    v2s = np.ascontiguousarray(W2 @ a2_src.T).astype(bf)
    in_maps = []
    for c in range(NCORES):
        sl = slice(c * SH, (c + 1) * SH)
        mm = (A[sl, :] > 0).T.astype(bf)
        mb = np.where(A[sl, :] > 0, 0.0, MB).T.astype(bf)
        srhs = np.zeros((128, SH), f)
        for h in range(H):
            srhs[32 * h, :] = 1.0
            srhs[32 * h + 1, :] = ssrcA[sl, h]
        in_maps.append({
            "h1b": h1b,
            "sdt2": sdt2,
            "srhs": srhs.astype(bf),
            "maskM": np.ascontiguousarray(mm),
            "maskB": np.ascontiguousarray(mb),
            "W2e": W2e,
            "v2s": v2s,
        })
    return in_maps


def kernel(x, A, W1, a1_src, a1_dst, W2, a2_src, a2_dst, _want_results=False):
    from concourse.bass_utils import run_bass_kernel_spmd

    nc = _get_nc()
    in_maps = _prep_in_maps(np.asarray(x), np.asarray(A), np.asarray(W1),
                            np.asarray(a1_src), np.asarray(a1_dst),
                            np.asarray(W2), np.asarray(a2_src),
                            np.asarray(a2_dst))
    trace = bool(int(os.environ.get("GAT_TRACE", "0")))
    res = run_bass_kernel_spmd(nc, in_maps, core_ids=list(range(NCORES)),
                               trace=trace)
    out = np.empty((N, C), np.float32)
    for c in range(NCORES):
        out[c * SH:(c + 1) * SH, :] = res.results[c]["outT"].T
    if _want_results:
        return out, res
    return out


# revision 29
# speedup vs baseline: 1.0071x; 1.0071x over previous
"""GAT (2-layer graph attention network) Bass kernel for 8 trn2 NeuronCores.

Sharding: core c owns node rows [512c, 512c+512). Weights replicated.
Scores live in transposed layout [j(partitions), i(free)] so the
aggregation matmul out1T[d', i] = sum_j h_aug[j, d'] * P[j, i] needs no
on-device transposes; the softmax denominator comes from a ones column
in the augmented feature matrix.

v5: h1/s_dst/s_src are host-precomputed. Layer-1 raw scores are built
ON THE TENSOR ENGINE as a K=5 matmul straight into PSUM:
  lhsT = [s_dst[j,0..3]; 1]  (5 x 128 chunk of host tensor)
  rhs  = [e_h blocks; s_src] (5 x H*SH constant)
  out[j,(h,i)] = s_dst[j,h] + s_src[h,i]
Exp (patched table: exp(lrelu)) reads the PSUM half-chunks directly and
the {0,1} mask applies as one bf16 multiply on DVE - so the vector
engine does a single op per chunk and never fights the PE for SBUF.
Layer 2 keeps the scalar_tensor_tensor form with an additive mask and
gathers the per-shard h2 (f32) in a blocked layout so the post-
collective reload is 8 contiguous DMAs.
"""

import os

import numpy as np

N, FIN, HID, H, D1, C = 4096, 512, 256, 4, 64, 64
NCORES = 8
SH = N // NCORES          # 512 local nodes per core
NB = N // 128             # 32 j-chunks
KC2 = HID // 128          # 2 hid chunks
NEG = 0.2                 # leaky relu slope
AUG = (D1 + 1) * H        # 260: per head [h (64) | 1]
MB = -30000.0             # additive mask value for non-edges (layer 2)
IB = SH // 128            # 4 local i-blocks

_CACHED = {}


def _make_act_root(alpha=NEG):
    """Patch the neuron ACT tables so Exp computes g(x)=exp(lrelu(x)).

    Bucket entries are [d0,d1,d2,d3,x0,0,0,0] fp32 cubics evaluated as
    y = d0+(x-x0)(d1+(x-x0)(d2+(x-x0)d3)). For exp buckets centered at
    x0<0 we substitute the Taylor cubic of exp(alpha*x) at the same
    center; the alpha contraction makes the cubic far more accurate than
    the original spline tolerance. Verified on HW: max rel err ~1.1e-5.
    """
    import json
    import shutil
    import tempfile

    from neuronxcc.driver.Job import Job
    from neuronxcc.driver.jobs.support.FindActInfo import findActInfoFile

    src_dir = os.path.dirname(findActInfoFile(Job.getPackageDir(), "gen3"))
    dst = tempfile.mkdtemp(prefix="gat_act_root_")
    for f in os.listdir(src_dir):
        shutil.copy(os.path.join(src_dir, f), os.path.join(dst, f))
        os.chmod(os.path.join(dst, f), 0o644)
    for set_name in ("exp_and_others", "natural_log_exp_and_others",
                     "exp_and_friends"):
        meta = json.load(open(os.path.join(dst, f"{set_name}.json")))
        start = meta["func_to_bkt_start_idx"].get("exp")
        if start is None:
            continue
        nxt = [s for s in sorted(meta["func_to_bkt_start_idx"].values())
               if s > start]
        end = nxt[0] if nxt else meta["bkt_entry_cnt"]
        path = os.path.join(dst, f"{set_name}_bkt.bin")
        b = np.fromfile(path, dtype=np.float32).reshape(-1, 8).copy()
        for i in range(start, end):
            x0, d0 = float(b[i, 4]), float(b[i, 0])
            if x0 >= 0 or not np.isfinite(d0) or d0 <= 0:
                continue
            e = np.exp(alpha * x0)
            b[i, 0:4] = [e, alpha * e, alpha * alpha * e / 2.0,
                         alpha ** 3 * e / 6.0]
        b.tofile(path)
    return os.path.join(dst, "act_info.json")


def _build_nc():
    os.environ["BASS_ACT_ROOT_JSON_PATH"] = _make_act_root()
    import concourse.mybir as mybir
    import concourse.tile as tile
    from concourse import bacc

    f32 = mybir.dt.float32
    bf16 = mybir.dt.bfloat16
    Af = mybir.ActivationFunctionType
    Alu = mybir.AluOpType

    nc = bacc.Bacc("TRN2", target_bir_lowering=False, debug=False,
                   num_devices=NCORES)

    # blocked host layouts: partition p holds j = 128*jc + p
    h1b_d = nc.dram_tensor("h1b", [128, NB * AUG], bf16,
                           kind="ExternalInput").ap()
    sdt2_d = nc.dram_tensor("sdt2", [128, N], bf16,
                            kind="ExternalInput").ap()
    srhs_d = nc.dram_tensor("srhs", [128, SH], bf16,
                            kind="ExternalInput").ap()
    mm_d = nc.dram_tensor("maskM", [N, SH], bf16, kind="ExternalInput").ap()
    mb_d = nc.dram_tensor("maskB", [N, SH], bf16, kind="ExternalInput").ap()
    W2e_d = nc.dram_tensor("W2e", [HID, C + 1], bf16, kind="ExternalInput").ap()
    v2s_d = nc.dram_tensor("v2s", [HID, 1], bf16, kind="ExternalInput").ap()
    outT_d = nc.dram_tensor("outT", [C, SH], f32, kind="ExternalOutput").ap()

    with tile.TileContext(nc) as tc:
        with tc.tile_pool(name="persist", bufs=1) as pp:
            h1aug = pp.tile([128, NB, AUG], bf16)
            mmul = pp.tile([128, NB, SH], bf16)     # {0,1} mask (layer 1)
            mbr = pp.tile([128, NB, SH], bf16)      # additive mask (layer 2)
            sdt2 = pp.tile([128, NB, 128], bf16)
            srhs = pp.tile([128, SH], bf16)
            z1Tl = pp.tile([128, KC2, SH], bf16)
            h2f = pp.tile([128, NB, C + 1], f32)    # gathered h2|s2dst
            h2s = pp.tile([128, NB, C + 1], bf16)   # bf16 stationary copy
            s2dstf = pp.tile([128, NB], f32)
            s2srcb = pp.tile([128, SH], bf16)
            s2srow = pp.tile([1, SH], bf16)
            W2sb = pp.tile([128, KC2, C + 1], bf16)
            v2sb = pp.tile([128, KC2, 1], bf16)

            # ---------- startup DMAs -----------------------------------
            nc.sync.dma_start(sdt2[:].rearrange("p n x -> p (n x)"), sdt2_d)
            nc.sync.dma_start(srhs[:], srhs_d)
            for kc in range(KC2):
                nc.sync.dma_start(W2sb[:, kc, :], W2e_d[kc * 128:(kc + 1) * 128, :])
                nc.sync.dma_start(v2sb[:, kc, :], v2s_d[kc * 128:(kc + 1) * 128, :])

            LOOK = 4  # chunks of DMA lookahead

            def fetch(nb):
                nc.sync.dma_start(h1aug[:, nb, :],
                                  h1b_d[:, nb * AUG:(nb + 1) * AUG])
                nc.sync.dma_start(mmul[:, nb, :],
                                  mm_d[nb * 128:(nb + 1) * 128, :])

            for i in range(LOOK):
                fetch(i)

            # ---------- layer 1: masked softmax + aggregation -----------
            with (tc.tile_pool(name="aggps", bufs=1, space="PSUM") as aggps,
                  tc.tile_pool(name="scps", bufs=2, space="PSUM") as scps,
                  tc.tile_pool(name="spool", bufs=3) as sp):
                o1 = aggps.tile([D1 + 1, H, SH], f32)

                for i in range(NB):
                    if i + LOOK < NB:
                        fetch(i + LOOK)
                    pex = sp.tile([128, H, SH], bf16, tag="pex")
                    pt = sp.tile([128, H, SH], bf16, tag="pt")
                    for half in range(2):
                        sc = scps.tile([128, 2 * SH], f32, tag="sc")
                        for hh in range(2):
                            h = 2 * half + hh
                            nc.tensor.matmul(
                                sc[:, hh * SH:(hh + 1) * SH],
                                sdt2[32 * h:32 * h + 2, i, :],
                                srhs[32 * h:32 * h + 2, :],
                                start=True, stop=True,
                                tile_position=(32 * h, 0))
                        nc.scalar.activation(
                            pex[:, 2 * half:2 * half + 2, :].rearrange(
                                "p h x -> p (h x)"),
                            sc[:], Af.Exp)
                        nc.vector.tensor_mul(
                            pt[:, 2 * half:2 * half + 2, :],
                            pex[:, 2 * half:2 * half + 2, :],
                            mmul[:, i, :].unsqueeze(1).to_broadcast(
                                (128, 2, SH)))
                        for hh in range(2):
                            h = 2 * half + hh
                            nc.tensor.matmul(
                                o1[:, h, :],
                                h1aug[:, i, (D1 + 1) * h:(D1 + 1) * (h + 1)],
                                pt[:, h, :],
                                start=(i == 0), stop=(i == NB - 1))

                # layer-2 additive mask loads overlap the tail/collective
                for i in range(NB):
                    nc.sync.dma_start(mbr[:, i, :],
                                      mb_d[i * 128:(i + 1) * 128, :])

                # ---- normalize + ELU -> z1Tl [256(=2x128), SH] ---------
                with tc.tile_pool(name="fin1", bufs=1) as fin:
                    drow = fin.tile([1, H, SH], f32, tag="drow")
                    nc.vector.tensor_copy(drow[:], o1[D1:D1 + 1, :, :])
                    denb = fin.tile([D1, H, SH], f32, tag="denb")
                    nc.gpsimd.partition_broadcast(
                        denb[:].rearrange("p h x -> p (h x)"),
                        drow[:].rearrange("p h x -> p (h x)"))
                    recb = fin.tile([D1, H, SH], f32, tag="recb")
                    nc.vector.reciprocal_approx_fast(
                        recb[:].rearrange("p h x -> p (h x)"),
                        denb[:].rearrange("p h x -> p (h x)"))
                    for h in range(H):
                        r0 = (h % 2) * D1
                        nc.vector.tensor_mul(z1Tl[r0:r0 + D1, h // 2, :],
                                             o1[0:D1, h, :], recb[:, h, :])
                    for kc in range(KC2):
                        r_ = fin.tile([128, SH], bf16, tag="relu")
                        m_ = fin.tile([128, SH], bf16, tag="minv")
                        e_ = fin.tile([128, SH], bf16, tag="expv")
                        nc.vector.tensor_scalar_max(r_[:], z1Tl[:, kc, :], 0.0)
                        nc.vector.tensor_scalar_min(m_[:], z1Tl[:, kc, :], 0.0)
                        nc.scalar.activation(e_[:], m_[:], Af.Exp, scale=5.0)
                        nc.vector.scalar_tensor_tensor(
                            z1Tl[:, kc, :], e_[:], -1.0, r_[:],
                            op0=Alu.add, op1=Alu.add)

            # ---------- local h2 shard + all-gather ---------------------
            with (tc.tile_pool(name="l2ps", bufs=2, space="PSUM") as l2ps,
                  tc.tile_pool(name="l2sb", bufs=1) as l2sb,
                  tc.tile_pool(name="dram", bufs=1, space="DRAM") as dpool):
                s2p = l2ps.tile([1, SH], f32, tag="s2p")
                for kc in range(KC2):
                    nc.tensor.matmul(s2p[:], v2sb[:, kc, :], z1Tl[:, kc, :],
                                     start=(kc == 0), stop=(kc == KC2 - 1))
                nc.vector.tensor_copy(s2srow[:], s2p[:])
                nc.gpsimd.partition_broadcast(s2srcb[:], s2srow[:])

                h2loc = l2sb.tile([128, IB, C + 1], f32, tag="h2loc")
                for ib in range(IB):
                    h2p = l2ps.tile([128, C + 1], f32, tag="h2p")
                    for kc in range(KC2):
                        nc.tensor.matmul(
                            h2p[:], z1Tl[:, kc, ib * 128:(ib + 1) * 128],
                            W2sb[:, kc, :],
                            start=(kc == 0), stop=(kc == KC2 - 1))
                    nc.vector.tensor_copy(h2loc[:, ib, :], h2p[:])

                # blocked gather payload: [128, IB*(C+1)] contiguous rows
                ag_in = dpool.tile([128, IB * (C + 1)], f32)
                ag_out = dpool.tile([NCORES * 128, IB * (C + 1)], f32,
                                    addr_space="Shared")
                nc.sync.dma_start(ag_in[:],
                                  h2loc[:].rearrange("p b c -> p (b c)"))
                nc.gpsimd.collective_compute(
                    "AllGather", Alu.bypass,
                    replica_groups=[list(range(NCORES))],
                    ins=[ag_in[:].opt()], outs=[ag_out[:].opt()])
                for r in range(NCORES):
                    nc.sync.dma_start(
                        h2f[:, IB * r:IB * (r + 1), :].rearrange(
                            "p b c -> p (b c)"),
                        ag_out[r * 128:(r + 1) * 128, :])
                nc.vector.tensor_copy(s2dstf[:, :], h2f[:, :, C])
                nc.vector.tensor_copy(h2s[:, :, 0:C], h2f[:, :, 0:C])
                nc.vector.memset(h2s[:, :, C:C + 1], 1.0)

            # ---------- layer 2: masked softmax + aggregation -----------
            with (tc.tile_pool(name="aggps2", bufs=1, space="PSUM") as aggps2,
                  tc.tile_pool(name="sp2", bufs=3) as sp2):
                o2 = aggps2.tile([C + 1, SH], f32)
                NQ = NB // 4
                for q in range(NQ):
                    squad = sp2.tile([128, 4, SH], bf16, tag="sq2")
                    pexq = sp2.tile([128, 4, SH], bf16, tag="px2")
                    for k in range(4):
                        i = 4 * q + k
                        nc.vector.scalar_tensor_tensor(
                            squad[:, k, :], s2srcb[:],
                            s2dstf[:, i:i + 1], mbr[:, i, :],
                            op0=Alu.add, op1=Alu.add)
                    nc.scalar.activation(
                        pexq[:].rearrange("p a x -> p (a x)"),
                        squad[:].rearrange("p a x -> p (a x)"),
                        Af.Exp)
                    for k in range(4):
                        i = 4 * q + k
                        nc.tensor.matmul(o2[:], h2s[:, i, :], pexq[:, k, :],
                                         start=(i == 0), stop=(i == NB - 1))

                with tc.tile_pool(name="fin2", bufs=1) as fin2:
                    drow2 = fin2.tile([1, SH], f32, tag="drow2")
                    nc.vector.tensor_copy(drow2[:], o2[C:C + 1, :])
                    denb2 = fin2.tile([C, SH], f32, tag="denb2")
                    nc.gpsimd.partition_broadcast(denb2[:], drow2[:])
                    recb2 = fin2.tile([C, SH], f32, tag="recb2")
                    nc.vector.reciprocal_approx_fast(recb2[:], denb2[:])
                    outsb = fin2.tile([C, SH], f32, tag="outsb")
                    nc.vector.tensor_mul(outsb[:], o2[0:C, :], recb2[:])
                    nc.sync.dma_start(outT_d, outsb[:])

    nc.compile()
    return nc


def _get_nc():
    if "nc" not in _CACHED:
        _CACHED["nc"] = _build_nc()
    return _CACHED["nc"]


def _prep_in_maps(x, A, W1, a1_src, a1_dst, W2, a2_src, a2_dst):
    import ml_dtypes
    bf = ml_dtypes.bfloat16
    f = np.float32
    x = x.astype(f, copy=False)
    W1r = W1.reshape(FIN, H, D1)
    V1s = np.einsum("fhd,hd->fh", W1r, a1_src).astype(f)
    V1d = np.einsum("fhd,hd->fh", W1r, a1_dst).astype(f)
    h1 = x @ W1                       # [N, HID]
    sdstA = x @ V1d                   # [N, H]
    ssrcA = x @ V1s                   # [N, H]
    h1aug = np.empty((N, H, D1 + 1), f)
    h1aug[:, :, 0:D1] = h1.reshape(N, H, D1)
    h1aug[:, :, D1] = 1.0
    h1b = np.ascontiguousarray(
        h1aug.reshape(NB, 128, AUG).transpose(1, 0, 2).reshape(128, NB * AUG)
    ).astype(bf)
    # row-tiled score matmuls: head h occupies PE rows 32h..32h+1 with
    # lhsT = [sdst[:,h]; 1] and rhs = [1; s_src[h,:]]
    sdt2 = np.zeros((128, N), f)
    for h in range(H):
        sdt2[32 * h, :] = sdstA[:, h]
        sdt2[32 * h + 1, :] = 1.0
    sdt2 = sdt2.astype(bf)
    W2e = np.ascontiguousarray(
        np.concatenate([W2, W2 @ a2_dst.T], axis=1)).astype(bf)
    v2s = np.ascontiguousarray(W2 @ a2_src.T).astype(bf)
    in_maps = []
    for c in range(NCORES):
        sl = slice(c * SH, (c + 1) * SH)
        mm = (A[sl, :] > 0).T.astype(bf)
        mb = np.where(A[sl, :] > 0, 0.0, MB).T.astype(bf)
        srhs = np.zeros((128, SH), f)
        for h in range(H):
            srhs[32 * h, :] = 1.0
            srhs[32 * h + 1, :] = ssrcA[sl, h]
        in_maps.append({
            "h1b": h1b,
            "sdt2": sdt2,
            "srhs": srhs.astype(bf),
            "maskM": np.ascontiguousarray(mm),
            "maskB": np.ascontiguousarray(mb),
            "W2e": W2e,
            "v2s": v2s,
        })
    return in_maps


def kernel(x, A, W1, a1_src, a1_dst, W2, a2_src, a2_dst, _want_results=False):
    from concourse.bass_utils import run_bass_kernel_spmd

    nc = _get_nc()
    in_maps = _prep_in_maps(np.asarray(x), np.asarray(A), np.asarray(W1),
                            np.asarray(a1_src), np.asarray(a1_dst),
                            np.asarray(W2), np.asarray(a2_src),
                            np.asarray(a2_dst))
    trace = bool(int(os.environ.get("GAT_TRACE", "0")))
    res = run_bass_kernel_spmd(nc, in_maps, core_ids=list(range(NCORES)),
                               trace=trace)
    out = np.empty((N, C), np.float32)
    for c in range(NCORES):
        out[c * SH:(c + 1) * SH, :] = res.results[c]["outT"].T
    if _want_results:
        return out, res
    return out
